# revision 2
# baseline (speedup 1.0000x reference)
# Trainium2 Bass kernel for nn_ExpertLinear (MoE grouped GEMM with routing).
#
# Strategy: data-parallel over tokens (8 cores), full weights replicated.
# Each core:
#   1. dma_gather(transpose=True): gather its token rows from HBM x (fp16)
#      directly into the transposed [d_in, tokens] stationary-operand layout,
#      grouped by expert (per-expert groups padded to multiples of 128 rows).
#   2. Grouped GEMM: per expert, per 128-row tile: accumulate 8 k-tiles into
#      PSUM via matmul(lhsT=xT tile, rhs=W[e] tile); evict PSUM with a
#      per-row gate multiply (DVE tensor_scalar) into an SBUF y buffer (fp16).
#   3. Combine: SBUF-source dma_gather (transpose mode) pulls each token's
#      two gated y rows; DVE add produces out^T (fp32); DMA to DRAM.
# Host assembles: de-transpose each core's out^T shard and scatter rows by
# the token->core assignment.
import os
import numpy as np

import concourse.bacc as bacc
import concourse.bass as bass
import concourse.mybir as mybir
import concourse.tile as tile
from concourse.bass_utils import run_bass_kernel_spmd

N_TOK = 8192
TOPK = 2
N_EXP = 8
D_IN = 1024
D_OUT = 1024
NCORES = 8
TPC = N_TOK // NCORES          # tokens per core
P = 128
KTILES = D_IN // P             # 8 k-tiles over d_in
F16 = mybir.dt.float16
F32 = mybir.dt.float32
I16 = mybir.dt.int16


def _plan(tok, sei, g_row):
    """Host routing plan. tok[i]: global token of grouped row i (expert-major
    order); sei[i]: its expert; g_row[i]: its gate. Returns the per-core
    index/gate tables and the shared tile counts T[e]."""
    n_rows = tok.shape[0]

    # experts of each token (k rows per token, in grouped-row order)
    order_by_tok = np.argsort(tok, kind="stable")
    te = sei[order_by_tok].reshape(N_TOK, TOPK)
    pair_id = te[:, 0] * N_EXP + te[:, 1]

    # balanced token->core assignment: round-robin inside expert-pair groups
    # with a global rotating counter => exactly TPC tokens/core and near-even
    # per-(core, expert) counts.
    order = np.argsort(pair_id, kind="stable")
    core_of_token = np.empty(N_TOK, np.int64)
    core_of_token[order] = np.arange(N_TOK) % NCORES

    token_ids = [np.where(core_of_token == c)[0] for c in range(NCORES)]
    token_pos = np.empty(N_TOK, np.int64)
    for c in range(NCORES):
        token_pos[token_ids[c]] = np.arange(TPC)

    core_of_row = core_of_token[tok]
    rows_per_core = [np.where(core_of_row == c)[0] for c in range(NCORES)]
    cnt = np.zeros((NCORES, N_EXP), np.int64)
    for c in range(NCORES):
        cnt[c] = np.bincount(sei[rows_per_core[c]], minlength=N_EXP)

    T = np.maximum(1, -(-cnt.max(axis=0) // P))       # tiles per expert
    off = np.concatenate([[0], np.cumsum(T) * P])      # padded row offsets
    NP = int(off[-1])

    per_core = []
    for c in range(NCORES):
        rows_c = rows_per_core[c]                      # ascending = expert-major
        e_c = sei[rows_c]
        # local padded slot of each real row: off[e] + rank within its expert
        within = np.arange(rows_c.size) - np.concatenate(
            [[0], np.cumsum(cnt[c])]
        )[e_c]
        loc = off[e_c] + within

        gidx_flat = np.zeros(NP, np.int16)
        grow_flat = np.zeros(NP, np.float32)
        gidx_flat[loc] = tok[rows_c].astype(np.int16)
        grow_flat[loc] = g_row[rows_c]

        # combine: first/second occurrence (in grouped order) per token
        pos = token_pos[tok[rows_c]]
        r0_flat = np.zeros(TPC, np.int16)
        r1_flat = np.zeros(TPC, np.int16)
        seen = np.zeros(TPC, bool)
        for j in range(rows_c.size):
            p_ = pos[j]
            if seen[p_]:
                r1_flat[p_] = loc[j]
            else:
                r0_flat[p_] = loc[j]
                seen[p_] = True
        assert seen.all()

        def pack16(flat):
            # [16, n/16] block (idx j at [j%16, j//16]), replicated into all
            # eight 16-partition groups — each GpSimd Q7 core reads its own
            # copy ("wrapped in 16 partitions and replicated across cores").
            return np.ascontiguousarray(np.tile(flat.reshape(-1, 16).T, (8, 1)))

        grow2d = np.ascontiguousarray(grow_flat.reshape(-1, P).T)
        per_core.append(
            dict(
                gidx=pack16(gidx_flat),
                grow=grow2d,
                r0i=pack16(r0_flat),
                r1i=pack16(r1_flat),
            )
        )
    return T, NP, per_core, token_ids


def _build_nc(T):
    NP = int(T.sum()) * P
    NB = NP // P                    # number of 128-row tiles
    Tmax = int(T.max())
    off = np.concatenate([[0], np.cumsum(T)]) * P

    nc = bacc.Bacc("TRN2", target_bir_lowering=False, debug=False,
                   num_devices=NCORES)

    xh = nc.dram_tensor("xh", [N_TOK, D_IN], F16, kind="ExternalInput")
    wh = nc.dram_tensor("wh", [N_EXP, P, KTILES, D_OUT], F16,
                        kind="ExternalInput")
    gidx = nc.dram_tensor("gidx", [P, NP // 16], I16, kind="ExternalInput")
    grow = nc.dram_tensor("grow", [P, NB], F32, kind="ExternalInput")
    r0i = nc.dram_tensor("r0i", [P, TPC // 16], I16, kind="ExternalInput")
    r1i = nc.dram_tensor("r1i", [P, TPC // 16], I16, kind="ExternalInput")
    outT = nc.dram_tensor("outT", [P, D_OUT // P, TPC], F32,
                          kind="ExternalOutput")

    CH = 512                         # combine chunk (tokens)

    with tile.TileContext(nc) as tc:
        with (
            tc.tile_pool(name="const", bufs=1) as kpool,
            tc.tile_pool(name="w", bufs=2) as wpool,
            tc.tile_pool(name="xT", bufs=3) as xpool,
            tc.tile_pool(name="y", bufs=1) as ypool,
            tc.tile_pool(name="cmb", bufs=2) as cpool,
            tc.tile_pool(name="ps", bufs=4, space="PSUM") as ppool,
        ):
            gidx_t = kpool.tile([P, NP // 16], I16)
            nc.sync.dma_start(gidx_t[:], gidx[:])
            grow_t = kpool.tile([P, NB], F32)
            nc.sync.dma_start(grow_t[:], grow[:])
            r0_t = kpool.tile([P, TPC // 16], I16)
            nc.sync.dma_start(r0_t[:], r0i[:])
            r1_t = kpool.tile([P, TPC // 16], I16)
            nc.sync.dma_start(r1_t[:], r1i[:])

            y_t = ypool.tile([P, NB, D_OUT], F16)

            rt_g = 0
            for e in range(N_EXP):
                ne = int(T[e]) * P
                w_t = wpool.tile([P, KTILES, D_OUT], F16, tag="w")
                nc.sync.dma_start(w_t[:], wh[e])
                x_t = xpool.tile([P, KTILES, ne], F16, tag="xT")
                nc.gpsimd.dma_gather(
                    x_t[:], xh[:],
                    gidx_t[:, off[e] // 16:(off[e] + ne) // 16],
                    num_idxs=ne, num_idxs_reg=ne, elem_size=D_IN,
                    transpose=True,
                )
                for t in range(int(T[e])):
                    ps0 = ppool.tile([P, 512], F32, tag="ps")
                    ps1 = ppool.tile([P, 512], F32, tag="ps")
                    for kk in range(KTILES):
                        lhsT = x_t[:, kk, t * P:(t + 1) * P]
                        nc.tensor.matmul(ps0[:], lhsT, w_t[:, kk, 0:512],
                                         start=(kk == 0), stop=(kk == KTILES - 1))
                        nc.tensor.matmul(ps1[:], lhsT, w_t[:, kk, 512:1024],
                                         start=(kk == 0), stop=(kk == KTILES - 1))
                    gsc = grow_t[:, rt_g:rt_g + 1]
                    nc.vector.tensor_scalar_mul(y_t[:, rt_g, 0:512], ps0[:], gsc)
                    nc.vector.tensor_scalar_mul(y_t[:, rt_g, 512:1024], ps1[:], gsc)
                    rt_g += 1

            for h in range(TPC // CH):
                y0 = cpool.tile([P, D_OUT // P, CH], F16, tag="y0")
                y1 = cpool.tile([P, D_OUT // P, CH], F16, tag="y1")
                for dst, ridx in ((y0, r0_t), (y1, r1_t)):
                    nc.gpsimd.dma_gather(
                        dst[:], y_t[:],
                        ridx[:, h * (CH // 16):(h + 1) * (CH // 16)],
                        num_idxs=CH, num_idxs_reg=CH, elem_size=D_OUT,
                        transpose=True,
                        sbuf_tokens_per_rank=P,
                        sbuf_free_dim_per_rank=D_OUT * 2,
                    )
                ot = cpool.tile([P, D_OUT // P, CH], F32, tag="ot")
                nc.vector.tensor_add(out=ot[:], in0=y0[:], in1=y1[:])
                nc.sync.dma_start(outT[:, :, h * CH:(h + 1) * CH], ot[:])

    nc.compile()
    return nc


def _prep(inputs):
    x = np.asarray(inputs["input"], np.float32)
    w = np.asarray(inputs["weight"], np.float32)
    k = int(np.asarray(inputs["k"]))
    assert k == TOPK
    sei = np.asarray(inputs["sorted_expert_indices"]).astype(np.int64)
    ssi = np.asarray(inputs["sorted_scattered_indices"]).astype(np.int64)
    gates = np.asarray(inputs["gates"], np.float32)

    tok = ssi // k
    g_row = gates.reshape(-1)[ssi]

    T, NP, per_core, token_ids = _plan(tok, sei, g_row)

    xh = x.astype(np.float16)
    wh = np.ascontiguousarray(
        w.reshape(N_EXP, KTILES, P, D_OUT).transpose(0, 2, 1, 3)
    ).astype(np.float16)

    in_maps = []
    for c in range(NCORES):
        m = dict(per_core[c])
        m["xh"] = xh
        m["wh"] = wh
        in_maps.append(m)
    return T, in_maps, token_ids


def _run(inputs, trace=False, trace_kwargs=None):
    T, in_maps, token_ids = _prep(inputs)
    nc = _build_nc(T)
    res = run_bass_kernel_spmd(
        nc, in_maps, core_ids=list(range(NCORES)), trace=trace,
        **(trace_kwargs or {}),
    )
    out = np.zeros((N_TOK, D_OUT), np.float32)
    for c in range(NCORES):
        oT = res.results[c]["outT"]            # [P, D_OUT//P, TPC]
        out[token_ids[c]] = oT.transpose(2, 1, 0).reshape(TPC, D_OUT)
    return out, res


def kernel(**inputs) -> np.ndarray:
    out, _ = _run(inputs, trace=bool(int(os.environ.get("KERNEL_TRACE", "0"))))
    return out


# revision 4
# speedup vs baseline: 1.0113x; 1.0113x over previous
# Trainium2 Bass kernel for nn_ExpertLinear (MoE grouped GEMM with routing).
#
# Strategy: data-parallel over tokens (8 cores), full weights replicated.
# Each core:
#   1. dma_gather(transpose=True): gather its token rows from HBM x (fp16)
#      directly into the transposed [d_in, tokens] stationary-operand layout,
#      grouped by expert (per-expert groups padded to multiples of 128 rows).
#   2. Grouped GEMM: per expert, per 128-row tile: accumulate 8 k-tiles into
#      PSUM via matmul(lhsT=xT tile, rhs=W[e] tile); evict PSUM with a
#      per-row gate multiply (DVE tensor_scalar) into an SBUF y buffer (fp16).
#   3. Combine: SBUF-source dma_gather (transpose mode) pulls each token's
#      two gated y rows; DVE add produces out^T (fp32); DMA to DRAM.
# Host assembles: de-transpose each core's out^T shard and scatter rows by
# the token->core assignment.
import os
import numpy as np

import concourse.bacc as bacc
import concourse.bass as bass
import concourse.mybir as mybir
import concourse.tile as tile
from concourse.bass_utils import run_bass_kernel_spmd

N_TOK = 8192
TOPK = 2
N_EXP = 8
D_IN = 1024
D_OUT = 1024
NCORES = 8
TPC = N_TOK // NCORES          # tokens per core
P = 128
KTILES = D_IN // P             # 8 k-tiles over d_in
F16 = mybir.dt.float16
F32 = mybir.dt.float32
I16 = mybir.dt.int16


def _plan(tok, sei, g_row):
    """Host routing plan. tok[i]: global token of grouped row i (expert-major
    order); sei[i]: its expert; g_row[i]: its gate. Returns the per-core
    index/gate tables and the shared tile counts T[e]."""
    n_rows = tok.shape[0]

    # experts of each token (k rows per token, in grouped-row order)
    order_by_tok = np.argsort(tok, kind="stable")
    te = sei[order_by_tok].reshape(N_TOK, TOPK)
    pair_id = te[:, 0] * N_EXP + te[:, 1]

    # balanced token->core assignment: round-robin inside expert-pair groups
    # with a global rotating counter => exactly TPC tokens/core and near-even
    # per-(core, expert) counts.
    order = np.argsort(pair_id, kind="stable")
    core_of_token = np.empty(N_TOK, np.int64)
    core_of_token[order] = np.arange(N_TOK) % NCORES

    token_ids = [np.where(core_of_token == c)[0] for c in range(NCORES)]
    token_pos = np.empty(N_TOK, np.int64)
    for c in range(NCORES):
        token_pos[token_ids[c]] = np.arange(TPC)

    core_of_row = core_of_token[tok]
    rows_per_core = [np.where(core_of_row == c)[0] for c in range(NCORES)]
    cnt = np.zeros((NCORES, N_EXP), np.int64)
    for c in range(NCORES):
        cnt[c] = np.bincount(sei[rows_per_core[c]], minlength=N_EXP)

    T = np.maximum(1, -(-cnt.max(axis=0) // P))       # tiles per expert
    off = np.concatenate([[0], np.cumsum(T) * P])      # padded row offsets
    NP = int(off[-1])

    per_core = []
    for c in range(NCORES):
        rows_c = rows_per_core[c]                      # ascending = expert-major
        e_c = sei[rows_c]
        # local padded slot of each real row: off[e] + rank within its expert
        within = np.arange(rows_c.size) - np.concatenate(
            [[0], np.cumsum(cnt[c])]
        )[e_c]
        loc = off[e_c] + within

        gidx_flat = np.zeros(NP, np.int16)
        grow_flat = np.zeros(NP, np.float32)
        gidx_flat[loc] = tok[rows_c].astype(np.int16)
        grow_flat[loc] = g_row[rows_c]

        # combine: first/second occurrence (in grouped order) per token
        pos = token_pos[tok[rows_c]]
        r0_flat = np.zeros(TPC, np.int16)
        r1_flat = np.zeros(TPC, np.int16)
        seen = np.zeros(TPC, bool)
        for j in range(rows_c.size):
            p_ = pos[j]
            if seen[p_]:
                r1_flat[p_] = loc[j]
            else:
                r0_flat[p_] = loc[j]
                seen[p_] = True
        assert seen.all()

        def pack16(flat):
            # [16, n/16] block (idx j at [j%16, j//16]), replicated into all
            # eight 16-partition groups — each GpSimd Q7 core reads its own
            # copy ("wrapped in 16 partitions and replicated across cores").
            return np.ascontiguousarray(np.tile(flat.reshape(-1, 16).T, (8, 1)))

        grow2d = np.ascontiguousarray(grow_flat.reshape(-1, P).T)
        per_core.append(
            dict(
                gidx=pack16(gidx_flat),
                grow=grow2d,
                r0i=pack16(r0_flat),
                r1i=pack16(r1_flat),
            )
        )
    return T, NP, per_core, token_ids


def _build_nc(T):
    NP = int(T.sum()) * P
    NB = NP // P                    # number of 128-row tiles
    Tmax = int(T.max())
    off = np.concatenate([[0], np.cumsum(T)]) * P

    nc = bacc.Bacc("TRN2", target_bir_lowering=False, debug=False,
                   num_devices=NCORES)

    xh = nc.dram_tensor("xh", [N_TOK, D_IN], F16, kind="ExternalInput")
    wh = nc.dram_tensor("wh", [N_EXP, P, KTILES, D_OUT], F16,
                        kind="ExternalInput")
    gidx = nc.dram_tensor("gidx", [P, NP // 16], I16, kind="ExternalInput")
    grow = nc.dram_tensor("grow", [P, NB], F32, kind="ExternalInput")
    r0i = nc.dram_tensor("r0i", [P, TPC // 16], I16, kind="ExternalInput")
    r1i = nc.dram_tensor("r1i", [P, TPC // 16], I16, kind="ExternalInput")
    outT = nc.dram_tensor("outT", [P, D_OUT // P, TPC], F32,
                          kind="ExternalOutput")

    CH = 256                         # combine chunk (tokens)
    NCH = TPC // CH

    with tile.TileContext(nc) as tc:
        with (
            tc.tile_pool(name="const", bufs=1) as kpool,
            tc.tile_pool(name="w", bufs=2) as wpool,
            tc.tile_pool(name="xT", bufs=1) as xpool,
            tc.tile_pool(name="y", bufs=1) as ypool,
            tc.tile_pool(name="cmb", bufs=1) as cpool,
            tc.tile_pool(name="ot", bufs=2) as opool,
            tc.tile_pool(name="ps", bufs=4, space="PSUM") as ppool,
        ):
            # index table first — the dispatch gathers depend only on it
            gidx_t = kpool.tile([P, NP // 16], I16)
            nc.sync.dma_start(gidx_t[:], gidx[:])

            # all dispatch gathers up front into dedicated tiles; SWDGE
            # desc-gen streams on GpSimd while W loads run on the ACT ring
            x_tiles = []
            for e in range(N_EXP):
                ne = int(T[e]) * P
                x_t = xpool.tile([P, KTILES, ne], F16, tag=f"x{e}")
                nc.gpsimd.dma_gather(
                    x_t[:], xh[:],
                    gidx_t[:, off[e] // 16:(off[e] + ne) // 16],
                    num_idxs=ne, num_idxs_reg=ne, elem_size=D_IN,
                    transpose=True,
                )
                x_tiles.append(x_t)

            grow_t = kpool.tile([P, NB], F32)
            nc.sync.dma_start(grow_t[:], grow[:])
            r0_t = kpool.tile([P, TPC // 16], I16)
            nc.sync.dma_start(r0_t[:], r0i[:])
            r1_t = kpool.tile([P, TPC // 16], I16)
            nc.sync.dma_start(r1_t[:], r1i[:])

            y_t = ypool.tile([P, NB, D_OUT], F16)

            rt_g = 0
            for e in range(N_EXP):
                w_t = wpool.tile([P, KTILES, D_OUT], F16, tag="w")
                nc.scalar.dma_start(w_t[:], wh[e])
                x_t = x_tiles[e]
                for t in range(int(T[e])):
                    ps0 = ppool.tile([P, 512], F32, tag="ps")
                    ps1 = ppool.tile([P, 512], F32, tag="ps")
                    for kk in range(KTILES):
                        lhsT = x_t[:, kk, t * P:(t + 1) * P]
                        nc.tensor.matmul(ps0[:], lhsT, w_t[:, kk, 0:512],
                                         start=(kk == 0), stop=(kk == KTILES - 1))
                        nc.tensor.matmul(ps1[:], lhsT, w_t[:, kk, 512:1024],
                                         start=(kk == 0), stop=(kk == KTILES - 1))
                    gsc = grow_t[:, rt_g:rt_g + 1]
                    nc.vector.tensor_scalar_mul(y_t[:, rt_g, 0:512], ps0[:], gsc)
                    nc.vector.tensor_scalar_mul(y_t[:, rt_g, 512:1024], ps1[:], gsc)
                    rt_g += 1

            # combine: chunked gather/add/store pipeline
            for h in range(NCH):
                y0 = cpool.tile([P, D_OUT // P, CH], F16, tag=f"c0_{h % 2}")
                y1 = cpool.tile([P, D_OUT // P, CH], F16, tag=f"c1_{h % 2}")
                for dst, ridx in ((y0, r0_t), (y1, r1_t)):
                    nc.gpsimd.dma_gather(
                        dst[:], y_t[:],
                        ridx[:, h * (CH // 16):(h + 1) * (CH // 16)],
                        num_idxs=CH, num_idxs_reg=CH, elem_size=D_OUT,
                        transpose=True,
                        sbuf_tokens_per_rank=P,
                        sbuf_free_dim_per_rank=D_OUT * 2,
                    )
                ot = opool.tile([P, D_OUT // P, CH], F32, tag="ot")
                nc.vector.tensor_add(out=ot[:], in0=y0[:], in1=y1[:])
                nc.sync.dma_start(outT[:, :, h * CH:(h + 1) * CH], ot[:])

    nc.compile()
    return nc


def _prep(inputs):
    x = np.asarray(inputs["input"], np.float32)
    w = np.asarray(inputs["weight"], np.float32)
    k = int(np.asarray(inputs["k"]))
    assert k == TOPK
    sei = np.asarray(inputs["sorted_expert_indices"]).astype(np.int64)
    ssi = np.asarray(inputs["sorted_scattered_indices"]).astype(np.int64)
    gates = np.asarray(inputs["gates"], np.float32)

    tok = ssi // k
    g_row = gates.reshape(-1)[ssi]

    T, NP, per_core, token_ids = _plan(tok, sei, g_row)

    xh = x.astype(np.float16)
    wh = np.ascontiguousarray(
        w.reshape(N_EXP, KTILES, P, D_OUT).transpose(0, 2, 1, 3)
    ).astype(np.float16)

    in_maps = []
    for c in range(NCORES):
        m = dict(per_core[c])
        m["xh"] = xh
        m["wh"] = wh
        in_maps.append(m)
    return T, in_maps, token_ids


def _run(inputs, trace=False, trace_kwargs=None):
    T, in_maps, token_ids = _prep(inputs)
    nc = _build_nc(T)
    res = run_bass_kernel_spmd(
        nc, in_maps, core_ids=list(range(NCORES)), trace=trace,
        **(trace_kwargs or {}),
    )
    out = np.zeros((N_TOK, D_OUT), np.float32)
    for c in range(NCORES):
        oT = res.results[c]["outT"]            # [P, D_OUT//P, TPC]
        out[token_ids[c]] = oT.transpose(2, 1, 0).reshape(TPC, D_OUT)
    return out, res


def kernel(**inputs) -> np.ndarray:
    out, _ = _run(inputs, trace=bool(int(os.environ.get("KERNEL_TRACE", "0"))))
    return out


# revision 6
# speedup vs baseline: 1.0541x; 1.0423x over previous
# Trainium2 Bass kernel for nn_ExpertLinear (MoE grouped GEMM with routing).
#
# Strategy: data-parallel over tokens (8 cores), full weights replicated.
# Each core:
#   1. dma_gather(transpose=True): gather its token rows from HBM x (fp16)
#      directly into the transposed [d_in, tokens] stationary-operand layout,
#      grouped by expert (per-expert groups padded to multiples of 128 rows).
#   2. Grouped GEMM: per expert, per 128-row tile: accumulate 8 k-tiles into
#      PSUM via matmul(lhsT=xT tile, rhs=W[e] tile); evict PSUM with a
#      per-row gate multiply (DVE tensor_scalar) into an SBUF y buffer (fp16).
#   3. Combine: SBUF-source dma_gather (transpose mode) pulls each token's
#      two gated y rows; DVE add produces out^T (fp32); DMA to DRAM.
# Host assembles: de-transpose each core's out^T shard and scatter rows by
# the token->core assignment.
import os
import numpy as np

import concourse.bacc as bacc
import concourse.bass as bass
import concourse.mybir as mybir
import concourse.tile as tile
from concourse.bass_utils import run_bass_kernel_spmd

N_TOK = 8192
TOPK = 2
N_EXP = 8
D_IN = 1024
D_OUT = 1024
NCORES = 8
TPC = N_TOK // NCORES          # tokens per core
P = 128
KTILES = D_IN // P             # 8 k-tiles over d_in
F16 = mybir.dt.float16
F32 = mybir.dt.float32
I16 = mybir.dt.int16


def _plan(tok, sei, g_row):
    """Host routing plan. tok[i]: global token of grouped row i (expert-major
    order); sei[i]: its expert; g_row[i]: its gate. Returns the per-core
    index/gate tables and the shared tile counts T[e]."""
    n_rows = tok.shape[0]

    # experts of each token (k rows per token, in grouped-row order)
    order_by_tok = np.argsort(tok, kind="stable")
    te = sei[order_by_tok].reshape(N_TOK, TOPK)
    pair_id = te[:, 0] * N_EXP + te[:, 1]

    # balanced token->core assignment: round-robin inside expert-pair groups
    # with a global rotating counter => exactly TPC tokens/core and near-even
    # per-(core, expert) counts.
    order = np.argsort(pair_id, kind="stable")
    core_of_token = np.empty(N_TOK, np.int64)
    core_of_token[order] = np.arange(N_TOK) % NCORES

    token_ids = [np.where(core_of_token == c)[0] for c in range(NCORES)]
    token_pos = np.empty(N_TOK, np.int64)
    for c in range(NCORES):
        token_pos[token_ids[c]] = np.arange(TPC)

    core_of_row = core_of_token[tok]
    rows_per_core = [np.where(core_of_row == c)[0] for c in range(NCORES)]
    cnt = np.zeros((NCORES, N_EXP), np.int64)
    for c in range(NCORES):
        cnt[c] = np.bincount(sei[rows_per_core[c]], minlength=N_EXP)

    T = np.maximum(1, -(-cnt.max(axis=0) // P))       # tiles per expert
    off = np.concatenate([[0], np.cumsum(T) * P])      # padded row offsets
    NP = int(off[-1])

    per_core = []
    for c in range(NCORES):
        rows_c = rows_per_core[c]                      # ascending = expert-major
        e_c = sei[rows_c]
        # local padded slot of each real row: off[e] + rank within its expert
        within = np.arange(rows_c.size) - np.concatenate(
            [[0], np.cumsum(cnt[c])]
        )[e_c]
        loc = off[e_c] + within

        gidx_flat = np.zeros(NP, np.int16)
        grow_flat = np.zeros(NP, np.float32)
        gidx_flat[loc] = tok[rows_c].astype(np.int16)
        grow_flat[loc] = g_row[rows_c]

        # combine: first/second occurrence (in grouped order) per token
        pos = token_pos[tok[rows_c]]
        r0_flat = np.zeros(TPC, np.int16)
        r1_flat = np.zeros(TPC, np.int16)
        seen = np.zeros(TPC, bool)
        for j in range(rows_c.size):
            p_ = pos[j]
            if seen[p_]:
                r1_flat[p_] = loc[j]
            else:
                r0_flat[p_] = loc[j]
                seen[p_] = True
        assert seen.all()

        def pack16(flat):
            # [16, n/16] block (idx j at [j%16, j//16]), replicated into all
            # eight 16-partition groups — each GpSimd Q7 core reads its own
            # copy ("wrapped in 16 partitions and replicated across cores").
            return np.ascontiguousarray(np.tile(flat.reshape(-1, 16).T, (8, 1)))

        grow2d = np.ascontiguousarray(grow_flat.reshape(-1, P).T)
        per_core.append(
            dict(
                gidx=pack16(gidx_flat),
                grow=grow2d,
                r0i=pack16(r0_flat),
                r1i=pack16(r1_flat),
            )
        )
    return T, NP, per_core, token_ids


def _build_nc(T):
    NP = int(T.sum()) * P
    NB = NP // P                    # number of 128-row tiles
    Tmax = int(T.max())
    off = np.concatenate([[0], np.cumsum(T)]) * P

    nc = bacc.Bacc("TRN2", target_bir_lowering=False, debug=False,
                   num_devices=NCORES)

    xh = nc.dram_tensor("xh", [N_TOK, D_IN], F16, kind="ExternalInput")
    wh = nc.dram_tensor("wh", [N_EXP, P, KTILES, D_OUT], F16,
                        kind="ExternalInput")
    gidx = nc.dram_tensor("gidx", [P, NP // 16], I16, kind="ExternalInput")
    grow = nc.dram_tensor("grow", [P, NB], F32, kind="ExternalInput")
    r0i = nc.dram_tensor("r0i", [P, TPC // 16], I16, kind="ExternalInput")
    r1i = nc.dram_tensor("r1i", [P, TPC // 16], I16, kind="ExternalInput")
    outT = nc.dram_tensor("outT", [P, D_OUT // P, TPC], F32,
                          kind="ExternalOutput")

    CH = 256                         # combine chunk (tokens)
    NCH = TPC // CH

    with tile.TileContext(nc) as tc:
        with (
            tc.tile_pool(name="const", bufs=1) as kpool,
            tc.tile_pool(name="w", bufs=2) as wpool,
            tc.tile_pool(name="xT", bufs=1) as xpool,
            tc.tile_pool(name="y", bufs=1) as ypool,
            tc.tile_pool(name="cmb", bufs=1) as cpool,
            tc.tile_pool(name="ot", bufs=2) as opool,
            tc.tile_pool(name="ps", bufs=4, space="PSUM") as ppool,
        ):
            # index table first — the dispatch gathers depend only on it
            gidx_t = kpool.tile([P, NP // 16], I16)
            nc.gpsimd.dma_start(gidx_t[:], gidx[:])

            # all dispatch gathers up front into dedicated tiles; SWDGE
            # desc-gen streams on GpSimd while W loads run on the ACT ring
            x_tiles = []
            for e in range(N_EXP):
                ne = int(T[e]) * P
                x_t = xpool.tile([P, KTILES, ne], F16, tag=f"x{e}")
                nc.gpsimd.dma_gather(
                    x_t[:], xh[:],
                    gidx_t[:, off[e] // 16:(off[e] + ne) // 16],
                    num_idxs=ne, num_idxs_reg=ne, elem_size=D_IN,
                    transpose=True,
                )
                x_tiles.append(x_t)

            grow_t = kpool.tile([P, NB], F32)
            nc.sync.dma_start(grow_t[:], grow[:])
            r0_t = kpool.tile([P, TPC // 16], I16)
            nc.sync.dma_start(r0_t[:], r0i[:])
            r1_t = kpool.tile([P, TPC // 16], I16)
            nc.sync.dma_start(r1_t[:], r1i[:])

            y_t = ypool.tile([P, NB, D_OUT], F16)

            rt_g = 0
            for e in range(N_EXP):
                w_t = wpool.tile([P, KTILES, D_OUT], F16, tag="w")
                for kk in range(KTILES):
                    nc.scalar.dma_start(w_t[:, kk], wh[e, :, kk])
                x_t = x_tiles[e]
                for t in range(int(T[e])):
                    ps0 = ppool.tile([P, 512], F32, tag="ps")
                    ps1 = ppool.tile([P, 512], F32, tag="ps")
                    for kk in range(KTILES):
                        lhsT = x_t[:, kk, t * P:(t + 1) * P]
                        nc.tensor.matmul(ps0[:], lhsT, w_t[:, kk, 0:512],
                                         start=(kk == 0), stop=(kk == KTILES - 1))
                        nc.tensor.matmul(ps1[:], lhsT, w_t[:, kk, 512:1024],
                                         start=(kk == 0), stop=(kk == KTILES - 1))
                    gsc = grow_t[:, rt_g:rt_g + 1]
                    nc.vector.tensor_scalar_mul(y_t[:, rt_g, 0:512], ps0[:], gsc)
                    nc.vector.tensor_scalar_mul(y_t[:, rt_g, 512:1024], ps1[:], gsc)
                    rt_g += 1

            # combine: chunked gather/add/store pipeline
            for h in range(NCH):
                y0 = cpool.tile([P, D_OUT // P, CH], F16, tag=f"c0_{h % 2}")
                y1 = cpool.tile([P, D_OUT // P, CH], F16, tag=f"c1_{h % 2}")
                for dst, ridx in ((y0, r0_t), (y1, r1_t)):
                    nc.gpsimd.dma_gather(
                        dst[:], y_t[:],
                        ridx[:, h * (CH // 16):(h + 1) * (CH // 16)],
                        num_idxs=CH, num_idxs_reg=CH, elem_size=D_OUT,
                        transpose=True,
                        sbuf_tokens_per_rank=P,
                        sbuf_free_dim_per_rank=D_OUT * 2,
                    )
                ot = opool.tile([P, D_OUT // P, CH], F32, tag="ot")
                nc.vector.tensor_add(out=ot[:], in0=y0[:], in1=y1[:])
                nc.sync.dma_start(outT[:, :, h * CH:(h + 1) * CH], ot[:])

    nc.compile()
    return nc


def _prep(inputs):
    x = np.asarray(inputs["input"], np.float32)
    w = np.asarray(inputs["weight"], np.float32)
    k = int(np.asarray(inputs["k"]))
    assert k == TOPK
    sei = np.asarray(inputs["sorted_expert_indices"]).astype(np.int64)
    ssi = np.asarray(inputs["sorted_scattered_indices"]).astype(np.int64)
    gates = np.asarray(inputs["gates"], np.float32)

    tok = ssi // k
    g_row = gates.reshape(-1)[ssi]

    T, NP, per_core, token_ids = _plan(tok, sei, g_row)

    xh = x.astype(np.float16)
    wh = np.ascontiguousarray(
        w.reshape(N_EXP, KTILES, P, D_OUT).transpose(0, 2, 1, 3)
    ).astype(np.float16)

    in_maps = []
    for c in range(NCORES):
        m = dict(per_core[c])
        m["xh"] = xh
        m["wh"] = wh
        in_maps.append(m)
    return T, in_maps, token_ids


def _run(inputs, trace=False, trace_kwargs=None):
    T, in_maps, token_ids = _prep(inputs)
    nc = _build_nc(T)
    res = run_bass_kernel_spmd(
        nc, in_maps, core_ids=list(range(NCORES)), trace=trace,
        **(trace_kwargs or {}),
    )
    out = np.zeros((N_TOK, D_OUT), np.float32)
    for c in range(NCORES):
        oT = res.results[c]["outT"]            # [P, D_OUT//P, TPC]
        out[token_ids[c]] = oT.transpose(2, 1, 0).reshape(TPC, D_OUT)
    return out, res


def kernel(**inputs) -> np.ndarray:
    out, _ = _run(inputs, trace=bool(int(os.environ.get("KERNEL_TRACE", "0"))))
    return out
